# revision 10
# baseline (speedup 1.0000x reference)
"""Self-contained Trainium2 Bass kernel for gated attention (sparse_attention).

Reference computation (per batch b):
    q = split_heads(x @ Wq) * DH**-0.5        # (H, n, DH)
    k, v = split_heads(x @ Wkv)               # (H, n, DH) each
    dots = q k^T + attn_bias ; masked softmax over j
    out = (attn @ v) reshaped to (n, H*DH)
    out = out * sigmoid(x @ Wg + bg)
    return out @ Wo + bo

Sharding: 8 cores = 4 batches x 2 query-row halves.  Each core projects
k/v only for its OWN 512 rows; halves are exchanged pairwise with
chunked AllGathers (DRAM bounce).  Per-core outputs are disjoint.

v4 design (one fully-pipelined TileContext):
  - x^T is precomputed on the host (no PE transposes); every DMA source
    is laid out p-major contiguous so descriptor generation is trivial.
  - j tiles are indexed LOGICALLY own-first: the own half of k/v is
    copied locally, the remote half lands via partition_id()-indexed
    DMA from the AllGather output.  Attention starts before the
    exchange completes.
  - attention per (dt, jo): QK pair (row groups) -> exp on ACT ->
    bias multiply on DVE (bf16 2x) -> AV per head with a 65th ones
    column in v that produces the softmax row-sums for free.
  - softmax normalization: PE broadcast of the sum rows, DVE
    reciprocal, gate folded in as a per-partition scalar when Wg == 0
    (the reference inits gating with weight=0 / bias=1).
"""
import sys
import types

import numpy as np
import ml_dtypes

# ---------------------------------------------------------------------------
# Environment shims (axon container): NTFF profile hook + walrus drain fix.
# ---------------------------------------------------------------------------


def _install_axon_ntff_hook():
    try:
        import antenv
    except ImportError:
        return
    if hasattr(antenv, "axon_hooks"):
        return
    mod = types.ModuleType("antenv.axon_hooks")
    mod._hook = None

    def set_axon_ntff_profile_hook(h):
        mod._hook = h

    def get_axon_ntff_profile_hook():
        return mod._hook

    mod.set_axon_ntff_profile_hook = set_axon_ntff_profile_hook
    mod.get_axon_ntff_profile_hook = get_axon_ntff_profile_hook
    sys.modules["antenv.axon_hooks"] = mod
    antenv.axon_hooks = mod
    try:
        from trn_agent_boot.trn_boot import _ntff_profile_via_ctypes

        hook = _ntff_profile_via_ctypes("/opt/axon/libaxon_pjrt.so")
        if hook is not None:
            set_axon_ntff_profile_hook(hook)
    except Exception:
        pass


_install_axon_ntff_hook()

import concourse.bass as bass  # noqa: E402
import concourse.tile as tile  # noqa: E402
import concourse.mybir as mybir  # noqa: E402
from concourse.bass_utils import run_bass_kernel_spmd  # noqa: E402
from concourse.tile import ScopedClock  # noqa: E402


def _patch_tile_drain():
    """The installed walrus accepts only one sync-wait per Drain; Tile's
    tail drain carries one wait per outstanding semaphore.  Split them
    across a chain of single-wait drains (same engine => same semantics)."""

    def _drain_and_barrier(self, tick_clock, wait_clock):
        nc = self.nc
        drain_inst = nc.sync.drain()
        wait_clock.add_sem_waits(
            drain_inst.ins, ScopedClock({None: tick_clock.global_clock})
        )
        si = drain_inst.ins.sync_info
        if si is not None and len(si.on_wait) > 1:
            waits = list(si.on_wait)
            drain_inst.ins.sync_info = mybir.SyncInfo(
                on_wait=waits[:1], on_update=list(si.on_update)
            )
            for w in waits[1:]:
                extra = nc.sync.drain()
                extra.ins.sync_info = mybir.SyncInfo(on_wait=[w], on_update=[])

        nc.all_engine_barrier()
        assert self.sems is not None
        popped = nc._tile_sem_poison_stack.pop()
        assert popped is self._sem_poison
        nc.clear_and_free_semaphores(list(self.sems.allocated().values()))
        nc.all_engine_barrier()

    tile.TileContext._drain_and_barrier = _drain_and_barrier


_patch_tile_drain()


def _legalize_waits(nc, max_waits=1):
    """Walrus in this container accepts at most one sync-wait per lowered
    instruction.  Move surplus waits onto single-wait NoOps inserted just
    before the instruction on the same engine."""
    nid = 0
    n_split = 0
    for f in nc.m.functions:
        for bb in f.blocks:
            out = []
            changed = False
            for inst in bb.instructions:
                si = inst.sync_info
                if si is not None and len(si.on_wait) > max_waits:
                    waits = list(si.on_wait)
                    for w in waits[:-1]:
                        nop = mybir.InstNoOp(name=f"WSPLIT-{nid}")
                        nid += 1
                        nop.engine = inst.engine
                        nop.sync_info = mybir.SyncInfo(on_wait=[w], on_update=[])
                        out.append(nop)
                    inst.sync_info = mybir.SyncInfo(
                        on_wait=[waits[-1]], on_update=list(si.on_update)
                    )
                    changed = True
                    n_split += 1
                out.append(inst)
            if changed:
                bb.instructions = out
    return n_split


# ---------------------------------------------------------------------------
# Problem constants (hardcoded per spec).
# ---------------------------------------------------------------------------
B, N, D = 4, 1024, 1024
H, DH = 8, 64
INNER = H * DH  # 512
M = N // 2  # 512 query rows per core
N_CORES = 8
P = 128
F32 = mybir.dt.float32
BF16 = mybir.dt.bfloat16

CT = D // P  # 8 contraction tiles over feature dim
DT = INNER // P  # 4 head pairs
NT = N // P  # 8 logical j tiles (own-first order)
JO = M // P  # 4 own j tiles
IB = M // P  # 4 tiles over query rows

Exp = mybir.ActivationFunctionType.Exp
MUL = mybir.AluOpType.mult
ADD = mybir.AluOpType.add


def _build_graph(gating: bool):
    nc = bass.Bass()
    xt_ext = nc.declare_dram_parameter("xt", [P, CT * M], BF16, isOutput=False)
    wk_ext = nc.declare_dram_parameter("wk", [P, CT * INNER], BF16, isOutput=False)
    wq_ext = nc.declare_dram_parameter("wq", [P, CT * INNER], BF16, isOutput=False)
    wv_ext = nc.declare_dram_parameter("wv", [P, CT * INNER], BF16, isOutput=False)
    if gating:
        wg_ext = nc.declare_dram_parameter("wg", [P, CT * INNER], BF16, isOutput=False)
        nbg_ext = nc.declare_dram_parameter("nbg", [P, DT], F32, isOutput=False)
    else:
        gc_ext = nc.declare_dram_parameter("gc", [P, DT], F32, isOutput=False)
    wo_ext = nc.declare_dram_parameter("wo", [P, DT * D], BF16, isOutput=False)
    bob_ext = nc.declare_dram_parameter("bob", [P, D], F32, isOutput=False)
    # bias layout: [dt*2 + half, p(=j within tile), jo(4), h(2), i(512)]
    bias_ext = nc.declare_dram_parameter(
        "bias", [DT * 2, P, JO * 2 * M], BF16, isOutput=False
    )
    out_ext = nc.declare_dram_parameter("out", [M, D], F32, isOutput=True)

    # remote-rank selector for the pairwise exchanges (graph is SPMD-uniform)
    pid = nc.sync.partition_id()
    rem = 1 - (pid % 2)

    PAIRS = [[0, 1], [2, 3], [4, 5], [6, 7]]

    with tile.TileContext(nc) as tc:
        with (
            tc.tile_pool(name="persist", bufs=1) as persist,
            tc.tile_pool(name="rings", bufs=1) as rings,
            tc.tile_pool(name="dram", bufs=1, space="DRAM") as dram,
            tc.tile_pool(name="ps", bufs=1, space="PSUM") as ps,
        ):
            # ---------------- persistent SBUF ----------------
            xT = persist.tile([P, CT, M], BF16, name="xT")
            kT = persist.tile([P, DT, N], BF16, name="kT")  # [dh-part, dt, j-logical]
            vA = persist.tile([P, 64, 64], BF16, name="vA")  # slot=(jo,dt,h)
            qT = persist.tile([P, DT, M], BF16, name="qT")
            gatedT = persist.tile([P, DT, M], BF16, name="gatedT")
            wk_sb = persist.tile([P, CT, INNER], BF16, name="wk_sb")
            wq_sb = persist.tile([P, CT, INNER], BF16, name="wq_sb")
            wv_sb = persist.tile([P, CT, INNER], BF16, name="wv_sb")
            if gating:
                wg_sb = persist.tile([P, CT, INNER], BF16, name="wg_sb")
                nbg_sb = persist.tile([P, DT], F32, name="nbg_sb")
                gT = persist.tile([P, DT, M], F32, name="gT")
            else:
                gc_sb = persist.tile([P, DT], F32, name="gc_sb")
            wo_sb = persist.tile([P, DT, D], BF16, name="wo_sb")
            bob_sb = persist.tile([P, D], F32, name="bob_sb")
            srow = persist.tile([P, 2, M], BF16, name="srow")  # sums @p0/p32
            ones_sb = persist.tile([P, P], BF16, name="ones_sb")
            warm_src = persist.tile([P, 512], BF16, name="warm_src")
            scr1 = persist.tile([P, 1], F32, name="scr1")

            bias_sb = {
                dt: persist.tile([P, NT, 2, M], BF16, name=f"bias{dt}")
                for dt in range(DT)
            }

            # DRAM bounce buffers for the chunked pairwise k/v AllGathers
            kb_in = [dram.tile([P, 2 * M], BF16, name=f"kbi{c}") for c in range(2)]
            kb_out = [dram.tile([2, P, 2 * M], BF16, name=f"kbo{c}") for c in range(2)]
            vb_in = [dram.tile([P, 16 * 64], BF16, name=f"vbi{c}") for c in range(2)]
            vb_out = [
                dram.tile([2, P, 16 * 64], BF16, name=f"vbo{c}") for c in range(2)
            ]

            # ---------------- init ----------------
            nc.gpsimd.memset(ones_sb, 1.0)
            nc.gpsimd.memset(warm_src, 1.0)

            # ---------------- DMA issue: sync ring (critical path) --------
            nc.sync.dma_start(out=xT.rearrange("p c m -> p (c m)"), in_=xt_ext[:])
            nc.sync.dma_start(out=wk_sb.rearrange("p c i -> p (c i)"), in_=wk_ext[:])
            nc.sync.dma_start(
                out=bias_sb[0].rearrange("p j h m -> p (j h m)")[:, 0 : JO * 2 * M],
                in_=bias_ext[0],
            )

            # ---------------- DMA issue: scalar ring ----------------------
            nc.scalar.dma_start(out=wq_sb.rearrange("p c i -> p (c i)"), in_=wq_ext[:])
            nc.scalar.dma_start(out=wv_sb.rearrange("p c i -> p (c i)"), in_=wv_ext[:])
            if gating:
                nc.scalar.dma_start(
                    out=wg_sb.rearrange("p c i -> p (c i)"), in_=wg_ext[:]
                )
                nc.scalar.dma_start(out=nbg_sb, in_=nbg_ext[:])
            else:
                nc.scalar.dma_start(out=gc_sb, in_=gc_ext[:])
            nc.scalar.dma_start(
                out=bias_sb[0].rearrange("p j h m -> p (j h m)")[:, JO * 2 * M :],
                in_=bias_ext[1],
            )
            for dt in range(1, DT):
                for half in range(2):
                    nc.scalar.dma_start(
                        out=bias_sb[dt].rearrange("p j h m -> p (j h m)")[
                            :, half * JO * 2 * M : (half + 1) * JO * 2 * M
                        ],
                        in_=bias_ext[dt * 2 + half],
                    )
            nc.scalar.dma_start(out=wo_sb.rearrange("p d i -> p (d i)"), in_=wo_ext[:])
            nc.scalar.dma_start(out=bob_sb, in_=bob_ext[:])

            # preload the ACT Exp table off the critical path
            nc.scalar.activation(out=scr1, in_=ones_sb[:, 0:1], func=Exp, scale=1.0)

            # ---------------- PE warmup ----------------
            for i in range(4):
                wt = ps.tile([P, 512], F32, tag="small", bufs=2, name="warm")
                nc.tensor.matmul(
                    wt, lhsT=warm_src[:, 0:P], rhs=warm_src, start=True, stop=True,
                    skip_group_check=True,
                )

            # ---------------- projection helpers ----------------
            def emit_kown(dt):
                pt = ps.tile([P, M], F32, tag="small", bufs=2, name="pk")
                for ct in range(CT):
                    nc.tensor.matmul(
                        pt,
                        lhsT=wk_sb[:, ct, dt * P : (dt + 1) * P],
                        rhs=xT[:, ct, :],
                        start=(ct == 0),
                        stop=(ct == CT - 1),
                    )
                nc.vector.tensor_copy(out=kT[:, dt, 0:M], in_=pt)

            def emit_vown(jo):
                pt = ps.tile([P, INNER], F32, tag="small", bufs=2, name="pv")
                for ct in range(CT):
                    nc.tensor.matmul(
                        pt,
                        lhsT=xT[:, ct, jo * P : (jo + 1) * P],
                        rhs=wv_sb[:, ct, :],
                        start=(ct == 0),
                        stop=(ct == CT - 1),
                    )
                nc.vector.tensor_copy(
                    out=vA[:, jo * 8 : (jo + 1) * 8, :].rearrange("p s e -> p (s e)"),
                    in_=pt,
                )

            def emit_qT(dt):
                pt = ps.tile([P, M], F32, tag="small", bufs=2, name="pq")
                for ct in range(CT):
                    nc.tensor.matmul(
                        pt,
                        lhsT=wq_sb[:, ct, dt * P : (dt + 1) * P],
                        rhs=xT[:, ct, :],
                        start=(ct == 0),
                        stop=(ct == CT - 1),
                    )
                nc.vector.tensor_copy(out=qT[:, dt, :], in_=pt)

            def emit_g(dt):
                pt = ps.tile([P, M], F32, tag="small", bufs=2, name="pg")
                for ct in range(CT):
                    nc.tensor.matmul(
                        pt,
                        lhsT=wg_sb[:, ct, dt * P : (dt + 1) * P],
                        rhs=xT[:, ct, :],
                        start=(ct == 0),
                        stop=(ct == CT - 1),
                    )
                # gT = 1 + exp(-(z + bg)); folded into the softmax recip later
                nc.scalar.activation(
                    out=gT[:, dt, :],
                    in_=pt,
                    func=Exp,
                    scale=-1.0,
                    bias=nbg_sb[:, dt : dt + 1],
                )
                nc.vector.tensor_scalar_add(gT[:, dt, :], gT[:, dt, :], 1.0)

            # ---------------- exchange helpers ----------------
            def emit_k_exchange(c):
                # k chunk c covers dt in {2c, 2c+1}; own half already in kT
                nc.sync.dma_start(
                    out=kb_in[c], in_=kT[:, 2 * c : 2 * c + 2, 0:M]
                )
                nc.gpsimd.collective_compute(
                    "AllGather",
                    mybir.AluOpType.bypass,
                    ins=[kb_in[c].opt()],
                    outs=[kb_out[c].opt()],
                    replica_groups=PAIRS,
                )
                nc.sync.dma_start(
                    out=kT[:, 2 * c : 2 * c + 2, M:N], in_=kb_out[c][rem]
                )

            def emit_v_exchange(c):
                # v chunk c covers logical jo in {2c, 2c+1} (16 slots, 65 wide)
                nc.sync.dma_start(
                    out=vb_in[c],
                    in_=vA[:, 16 * c : 16 * (c + 1), :].rearrange(
                        "p s e -> p (s e)"
                    ),
                )
                nc.gpsimd.collective_compute(
                    "AllGather",
                    mybir.AluOpType.bypass,
                    ins=[vb_in[c].opt()],
                    outs=[vb_out[c].opt()],
                    replica_groups=PAIRS,
                )
                nc.sync.dma_start(
                    out=vA[:, 32 + 16 * c : 48 + 16 * c, :].rearrange(
                        "p s e -> p (s e)"
                    ),
                    in_=vb_out[c][rem],
                )

            # ---------------- attention helpers ----------------
            aT_tiles = {}
            pav_tiles = {}

            def emit_qk(dt, jo):
                pd = ps.tile([P, 2, M], F32, tag="pd", bufs=2, name="pd")
                for hi in range(2):
                    po = 64 * hi
                    nc.tensor.matmul(
                        pd[:, hi, :],
                        lhsT=kT[po : po + 64, dt, jo * P : (jo + 1) * P],
                        rhs=qT[po : po + 64, dt, :],
                        start=True,
                        stop=True,
                    )
                aT = aT_tiles[dt]
                dst = aT[:, jo * 2 * M : (jo + 1) * 2 * M]
                nc.scalar.activation(
                    out=dst, in_=pd.rearrange("p a b -> p (a b)"), func=Exp, scale=1.0
                )
                nc.vector.tensor_tensor(
                    dst,
                    dst,
                    bias_sb[dt]
                    .rearrange("p j h m -> p (j h m)")[
                        :, jo * 2 * M : (jo + 1) * 2 * M
                    ],
                    MUL,
                )

            sums_tiles = {}

            def emit_av(dt, jo):
                # AV pair on disjoint col groups (concurrent), plus two M=1
                # row-sum matmuls into a shared PSUM bank (partitions 0/32).
                if jo == 0:
                    pav_tiles[dt] = ps.tile([P, M], F32, tag="pav", bufs=1, name="pav")
                    sums_tiles[dt] = ps.tile(
                        [33, M], F32, tag="sums", bufs=1, name="sums"
                    )
                st, sp = (jo == 0), (jo == NT - 1)
                s0 = jo * 8 + dt * 2
                aT = aT_tiles[dt]
                pav = pav_tiles[dt]
                sums = sums_tiles[dt]
                rhs0 = aT[:, (jo * 2) * M : (jo * 2 + 1) * M]
                rhs1 = aT[:, (jo * 2 + 1) * M : (jo * 2 + 2) * M]
                nc.tensor.matmul(
                    pav[0:64, :], lhsT=vA[:, s0, :], rhs=rhs0,
                    start=st, stop=sp, skip_group_check=True,
                )
                nc.tensor.matmul(
                    pav[64:128, :], lhsT=vA[:, s0 + 1, :], rhs=rhs1,
                    start=st, stop=sp, tile_position=(0, 64), skip_group_check=True,
                )
                nc.tensor.matmul(
                    sums[0:1, :], lhsT=ones_sb[:, 0:1], rhs=rhs0,
                    start=st, stop=sp, skip_group_check=True,
                )
                nc.tensor.matmul(
                    sums[32:33, :], lhsT=ones_sb[:, 0:1], rhs=rhs1,
                    start=st, stop=sp, tile_position=(0, 32), skip_group_check=True,
                )

            def emit_norm_gate(dt):
                pav = pav_tiles.pop(dt)
                sums = sums_tiles.pop(dt)
                # stage sum rows into SBUF for the broadcast MMs
                nc.vector.tensor_copy(out=srow[0:1, 0, :], in_=sums[0:1, :])
                nc.vector.tensor_copy(out=srow[32:33, 1, :], in_=sums[32:33, :])
                prf = ps.tile([P, M], F32, tag="small", bufs=2, name="prf")
                nc.tensor.matmul(
                    prf[0:64, :],
                    lhsT=ones_sb[0:1, 0:64],
                    rhs=srow[0:1, 0, :],
                    start=True,
                    stop=True,
                    skip_group_check=True,
                )
                nc.tensor.matmul(
                    prf[64:128, :],
                    lhsT=ones_sb[32:33, 0:64],
                    rhs=srow[32:33, 1, :],
                    start=True,
                    stop=True,
                    tile_position=(32, 64),
                    skip_group_check=True,
                )
                geff = rings.tile([P, M], F32, tag="geff", bufs=2, name="geff")
                if gating:
                    # geff = gate / sum = 1 / (sum * (1 + exp(-(z+bg))))
                    nc.vector.tensor_tensor(geff, prf, gT[:, dt, :], MUL)
                    nc.vector.reciprocal(geff, geff)
                else:
                    grec = rings.tile([P, M], F32, tag="grec", bufs=2, name="grec")
                    nc.vector.reciprocal(grec, prf)
                    nc.vector.tensor_scalar_mul(geff, grec, gc_sb[:, dt : dt + 1])
                nc.vector.tensor_tensor(
                    gatedT[:, dt, :], pav, geff, MUL
                )

            # ---------------- main pipeline ----------------
            emit_kown(0)
            emit_kown(1)
            emit_k_exchange(0)
            emit_kown(2)
            emit_kown(3)
            emit_k_exchange(1)
            emit_vown(0)
            emit_vown(1)
            emit_v_exchange(0)
            emit_qT(0)

            aT_tiles[0] = rings.tile([P, NT * 2 * M], BF16, tag="aT", bufs=2, name="aT")
            for jo in range(4):
                emit_qk(0, jo)

            emit_vown(2)
            emit_vown(3)
            emit_v_exchange(1)
            emit_qT(1)
            if gating:
                emit_g(0)
                emit_g(1)

            for jo in range(4, NT):
                emit_qk(0, jo)
                emit_av(0, jo - 4)
            emit_qT(2)
            for jo in range(4, NT):
                emit_av(0, jo)
            emit_norm_gate(0)

            aT_tiles[1] = rings.tile([P, NT * 2 * M], BF16, tag="aT", bufs=2, name="aT")
            for jo in range(4):
                emit_qk(1, jo)
                if jo >= 2:
                    emit_av(1, jo - 2)
            emit_qT(3)
            if gating:
                emit_g(2)
                emit_g(3)
            for jo in range(4, NT):
                emit_qk(1, jo)
                emit_av(1, jo - 2)
            emit_av(1, NT - 2)
            emit_av(1, NT - 1)
            emit_norm_gate(1)

            for dt in range(2, DT):
                aT_tiles[dt] = rings.tile(
                    [P, NT * 2 * M], BF16, tag="aT", bufs=2, name="aT"
                )
                for jo in range(NT):
                    emit_qk(dt, jo)
                    if jo >= 2:
                        emit_av(dt, jo - 2)
                emit_av(dt, NT - 2)
                emit_av(dt, NT - 1)
                emit_norm_gate(dt)

            # ---------------- output projection ----------------
            for ib in range(IB):
                po = ps.tile([P, 2, 512], F32, tag="pd", bufs=2, name="po")
                for dt in range(DT):
                    for dh in range(2):
                        nc.tensor.matmul(
                            po[:, dh, :],
                            lhsT=gatedT[:, dt, ib * P : (ib + 1) * P],
                            rhs=wo_sb[:, dt, dh * 512 : (dh + 1) * 512],
                            start=(dt == 0),
                            stop=(dt == DT - 1),
                            skip_group_check=True,
                        )
                osb = rings.tile([P, D], F32, tag="osb", bufs=2, name="osb")
                nc.vector.tensor_tensor(
                    osb, po.rearrange("p a b -> p (a b)"), bob_sb, ADD
                )
                nc.sync.dma_start(
                    out=out_ext.rearrange("(ib p) d -> p ib d", p=P)[:, ib, :],
                    in_=osb,
                )

    _legalize_waits(nc)
    return nc


_NC_CACHE = {}


def _get_graph(gating: bool):
    if gating not in _NC_CACHE:
        _NC_CACHE[gating] = _build_graph(gating)
    return _NC_CACHE[gating]


def _prepare_in_maps(x, mask, attn_bias, Wq, Wkv, Wg, bg, Wo, bo):
    x = np.asarray(x, dtype=np.float32)
    mask = np.asarray(mask, dtype=bool)
    attn_bias = np.asarray(attn_bias, dtype=np.float32)
    Wq = np.asarray(Wq, dtype=np.float32)
    Wkv = np.asarray(Wkv, dtype=np.float32)
    Wg = np.asarray(Wg, dtype=np.float32)
    bg = np.asarray(bg, dtype=np.float32)
    Wo = np.asarray(Wo, dtype=np.float32)
    bo = np.asarray(bo, dtype=np.float32)

    gating = bool(np.any(Wg != 0.0))

    def pmajor(w):
        # [D, C] -> [P, D//P, C] p-major contiguous
        c = w.shape[1]
        return np.ascontiguousarray(
            w.reshape(-1, P, c).transpose(1, 0, 2).reshape(P, -1)
        ).astype(ml_dtypes.bfloat16)

    wq_h = pmajor(Wq * np.float32(DH**-0.5))
    wk_h = pmajor(Wkv[:, :INNER])
    wv_h = pmajor(Wkv[:, INNER:])
    wo_h = pmajor(Wo)
    bob = np.ascontiguousarray(np.broadcast_to(bo.reshape(1, D), (P, D))).astype(
        np.float32
    )
    if gating:
        wg_h = pmajor(Wg)
        nbg = np.ascontiguousarray(-bg.reshape(DT, P).T).astype(np.float32)
    else:
        gc = np.ascontiguousarray(
            (1.0 / (1.0 + np.exp(-bg))).reshape(DT, P).T
        ).astype(np.float32)

    # Fold the attention mask into the bias (both sides), then exponentiate:
    # the kernel computes attn = exp(qk) * exp(bias).  Masked entries -> 0.
    m2 = mask[:, None, :, None] & mask[:, None, None, :]  # (B, 1, n, n)
    bias_eff = np.where(m2, attn_bias, np.float32(-np.inf))
    bias_eff = np.exp(bias_eff)

    in_maps = []
    for c in range(N_CORES):
        b, r = divmod(c, 2)
        xo = x[b, r * M : (r + 1) * M]  # (M, D) own rows
        xt = np.ascontiguousarray(
            xo.T.reshape(CT, P, M).transpose(1, 0, 2).reshape(P, CT * M)
        ).astype(ml_dtypes.bfloat16)
        # bias slice: own query rows i, all j; logical j order = own-first
        bc = bias_eff[b][:, r * M : (r + 1) * M, :]  # (H, M, N)
        # -> [dt, jt_phys, p, h, i]
        bt = bc.reshape(DT, 2, M, NT, P).transpose(0, 3, 4, 1, 2)
        jts = [r * JO + t for t in range(JO)] + [
            (1 - r) * JO + t for t in range(JO)
        ]
        bt = bt[:, jts]  # own-first logical order
        bt = np.ascontiguousarray(
            bt.reshape(DT, 2, JO, P, 2, M)
            .transpose(0, 1, 3, 2, 4, 5)
            .reshape(DT * 2, P, JO * 2 * M)
        ).astype(ml_dtypes.bfloat16)
        m = {
            "xt": xt,
            "wk": wk_h,
            "wq": wq_h,
            "wv": wv_h,
            "wo": wo_h,
            "bob": bob,
            "bias": bt,
        }
        if gating:
            m["wg"] = wg_h
            m["nbg"] = nbg
        else:
            m["gc"] = gc
        in_maps.append(m)
    return in_maps, gating


def _assemble(results):
    out = np.empty((B, N, D), dtype=np.float32)
    for c in range(N_CORES):
        b, r = divmod(c, 2)
        out[b, r * M : (r + 1) * M, :] = results[c]["out"]
    return out


def _run(in_maps, gating, trace=False):
    nc = _get_graph(gating)
    last_err = None
    for attempt in range(3):
        try:
            return run_bass_kernel_spmd(
                nc, in_maps, core_ids=list(range(N_CORES)), trace=trace
            )
        except Exception as e:  # transient device faults recover on retry
            last_err = e
    raise last_err


def kernel(**inputs):
    in_maps, gating = _prepare_in_maps(**inputs)
    res = _run(in_maps, gating)
    return _assemble(res.results)


def kernel_traced(**inputs):
    """Like kernel() but with NTFF profiling; returns (out, exec_time_ns)."""
    in_maps, gating = _prepare_in_maps(**inputs)
    res = _run(in_maps, gating, trace=True)
    return _assemble(res.results), res.exec_time_ns
